# revision 5
# baseline (speedup 1.0000x reference)
"""Positional-embedding lookup kernel for TRN2 (8 NeuronCores).

out[b, l, :] = emb_table[input_ids[b, l], :] + pe[l, :]

Sharding: data-parallel over batch (batch == n_cores == 8). Core b handles
batch row b: indirect-DMA gathers 4096 rows of 1024 f32 from the replicated
table, adds the (host-precomputed) sinusoidal positional encoding on DVE,
and stores its [4096, 1024] output slice.

Raw Bass (not Tile): the per-instruction sync-wait slots on this compiler
are too few for the gather pipeline, so semaphores are managed manually via
standalone wait_ge instructions.
"""

import numpy as np

import concourse.bass as bass
import concourse.mybir as mybir
from concourse.bass_utils import run_bass_kernel_spmd

VOCAB = 50257
D = 1024
BATCH = 8
SEQ = 4096
P = 128
N_TILES = SEQ // P  # 32
N_CORES = 8
B = 8  # tok ring buffers

_CACHED = {}


def _sinusoidal_table(seq_len: int, d_model: int) -> np.ndarray:
    i = np.arange(0, d_model // 2, dtype=np.float32)
    pos = np.arange(seq_len, dtype=np.float32)[:, None]
    div = pos / np.power(np.float32(10000.0), 2.0 * i / np.float32(d_model))
    pe = np.stack((np.sin(div), np.cos(div)), axis=2).reshape(seq_len, -1)
    return np.ascontiguousarray(pe[:, :d_model], dtype=np.float32)


def _build_nc():
    nc = bass.Bass("TRN2")
    ids_t = nc.dram_tensor("ids", [P, N_TILES], mybir.dt.int32, kind="ExternalInput")
    table_t = nc.dram_tensor(
        "table", [VOCAB, D], mybir.dt.float32, kind="ExternalInput"
    )
    pe_t = nc.dram_tensor(
        "pe", [P, N_TILES * D], mybir.dt.float32, kind="ExternalInput"
    )
    out_t = nc.dram_tensor("out", [SEQ, D], mybir.dt.float32, kind="ExternalOutput")

    import contextlib

    with contextlib.ExitStack() as ctx:
        ids_sb = ctx.enter_context(
            nc.sbuf_tensor("ids_sb", [P, N_TILES], mybir.dt.int32)
        )
        pe_sb = ctx.enter_context(
            nc.sbuf_tensor("pe_sb", [P, N_TILES * D], mybir.dt.float32)
        )
        tok_sb = ctx.enter_context(
            nc.sbuf_tensor("tok_sb", [P, B * D], mybir.dt.float32)
        )
        s_ids = ctx.enter_context(nc.semaphore("s_ids"))
        s_pe = ctx.enter_context(nc.semaphore("s_pe"))
        s_a = ctx.enter_context(nc.semaphore("s_a"))
        # per-buffer-slot DMA completion sems: same-slot DMAs are serialized
        # by the pipeline, so per-slot counting is race-free even though the
        # 16 SDMA engines complete different DMAs with arbitrary skew
        s_g = [ctx.enter_context(nc.semaphore(f"s_g{j}")) for j in range(B)]
        s_st = [ctx.enter_context(nc.semaphore(f"s_st{j}")) for j in range(B)]
        block = ctx.enter_context(nc.Block())

        @block.sync
        def _(sync):
            sync.dma_start(ids_sb[:], ids_t[:]).then_inc(s_ids, 16)
            sync.dma_start(pe_sb[:], pe_t[:]).then_inc(s_pe, 16)
            for i in range(N_TILES):
                sync.wait_ge(s_a, i + 1)
                sl = slice((i % B) * D, (i % B + 1) * D)
                sync.dma_start(out_t[i * P : (i + 1) * P, :], tok_sb[:, sl]).then_inc(
                    s_st[i % B], 16
                )
            for j in range(B):
                sync.wait_ge(s_st[j], 16 * (N_TILES // B))

        @block.gpsimd
        def _(g):
            g.wait_ge(s_ids, 16)
            for i in range(N_TILES):
                if i >= B:
                    # tok slot reuse: store of iteration i-B must be done
                    g.wait_ge(s_st[i % B], 16 * (i // B))
                sl = slice((i % B) * D, (i % B + 1) * D)
                g.indirect_dma_start(
                    out=tok_sb[:, sl],
                    out_offset=None,
                    in_=table_t[:],
                    in_offset=bass.IndirectOffsetOnAxis(
                        ap=ids_sb[:, i : i + 1], axis=0
                    ),
                ).then_inc(s_g[i % B], 16)

        @block.vector
        def _(v):
            v.wait_ge(s_pe, 16)
            for i in range(N_TILES):
                v.wait_ge(s_g[i % B], 16 * (i // B + 1))
                sl = slice((i % B) * D, (i % B + 1) * D)
                v.tensor_add(
                    out=tok_sb[:, sl],
                    in0=tok_sb[:, sl],
                    in1=pe_sb[:, i * D : (i + 1) * D],
                ).then_inc(s_a, 1)

    return nc


def kernel(input_ids, emb_table, encoder_seq_L):
    ids = np.ascontiguousarray(np.asarray(input_ids)).astype(np.int32)
    table = np.ascontiguousarray(np.asarray(emb_table), dtype=np.float32)
    assert ids.shape == (BATCH, SEQ)
    assert table.shape == (VOCAB, D)

    if "nc" not in _CACHED:
        _CACHED["nc"] = _build_nc()
        pe = _sinusoidal_table(SEQ, D)
        # pe_re[p, i*D + c] = pe[i*P + p, c]
        _CACHED["pe"] = np.ascontiguousarray(
            pe.reshape(N_TILES, P, D).transpose(1, 0, 2).reshape(P, N_TILES * D)
        )
    nc = _CACHED["nc"]
    pe_re = _CACHED["pe"]

    in_maps = []
    for b in range(N_CORES):
        # column i of ids_re holds tokens for positions i*128 .. i*128+127
        ids_re = np.ascontiguousarray(ids[b].reshape(N_TILES, P).T)
        in_maps.append({"ids": ids_re, "table": table, "pe": pe_re})

    res = run_bass_kernel_spmd(nc, in_maps, core_ids=list(range(N_CORES)))
    out = np.stack([r["out"] for r in res.results], axis=0)
    return out
